# revision 1
# baseline (speedup 1.0000x reference)
"""T5-style MultiHeadAttention (relative position bias) on 8 Trainium2 cores.

Sharding: core c = (b, g) with b = c // 4 (batch), g = c % 4 (head group of 4
heads).  Each core computes q/k/v projections for its 4 heads, attention with
the relative-position bias, and a partial output projection (rows of Wo for
its heads).  Host sums the 4 partials per batch element.

Key layout choices (per core):
  - x is passed transposed: xT [1024, 2048] so projections contract over
    partitions directly.
  - Q_t, K_t stored as [d, seq] (d on partitions); scores computed
    *transposed* as S_t[k, q] = K_t^T-stationary matmul, so that exp(S_t) can
    be used directly as the stationary operand of the attn@V matmul (which
    contracts over k = partitions).
  - Softmax denominator Z[q] = sum_k exp(S_t[k, q]) falls out of the attn@V
    matmul for free via a ones-column appended to V (M=65 per head).
  - No max-subtraction: scores are O(50) at most, exp stays finite in fp32
    and bf16.
  - Relative-position bias applied multiplicatively after exp:
    exp(s + bias) = exp(s) * exp(bias).  bias[k, q] = v_h[k - q + 2047] is
    Toeplitz, so exp(bias) tiles are overlapping windows of a single
    [128, 3968] sliding table per head, precomputed on host:
      T_h[p, i] = exp(v_h[p + 3967 - i])
    and the tile for (k0 = kc*128, q0 = qb*512) is T_h[:, base:base+512] with
    base = 1920 - k0 + q0.
  - Matmuls run in float32r (full PE rate at N>=256); the attention
    probabilities / V use bf16 (configurable) for the 2x DVE multiply mode.
"""

import numpy as np
from contextlib import ExitStack

import concourse.bass as bass
import concourse.tile as tile
from concourse import bacc, mybir
from concourse.bass_utils import run_bass_kernel_spmd

# ---------------------------------------------------------------- constants
B, S, D_MODEL, N_HEADS, D_KV = 2, 2048, 1024, 16, 64
NUM_BUCKETS, MAX_DIST = 32, 128
N_CORES = 8
HPC = N_HEADS // (N_CORES // B)  # heads per core = 4
DH = HPC * D_KV                  # 256 d-cols per core
TBL = 3968                       # exp-bias sliding table width
QB = 512                         # q block (free dim of score tiles)
KC = 128                         # k chunk (partition dim of score tiles)

F32 = mybir.dt.float32
F32R = mybir.dt.float32r
BF16 = mybir.dt.bfloat16
AF = mybir.ActivationFunctionType

# attention-probability dtype: BF16 (fast DVE 2x) or F32 (accurate, 1x DVE)
ATT_DT = BF16

_cache = {}


# ------------------------------------------------------------- host helpers
def _rel_bucket(d):
    """Bucket of relative position d = k - q (bidirectional T5), numpy fp32
    mirror of the jax reference."""
    nb = NUM_BUCKETS // 2
    n = -d
    ret = (n < 0).astype(np.int32) * nb
    n = np.abs(n)
    max_exact = nb // 2
    is_small = n < max_exact
    nf = np.maximum(n, 1).astype(np.float32)
    val = (
        np.log(nf / np.float32(max_exact))
        / np.float32(np.log(MAX_DIST / max_exact))
        * np.float32(nb - max_exact)
    ).astype(np.int32) + max_exact
    val = np.minimum(val, nb - 1)
    return ret + np.where(is_small, n, val)


def _expbias_tables(rel_emb):
    """[N_HEADS, 128, TBL] exp-bias sliding tables (float32)."""
    d = np.arange(-(S - 1), S)  # k - q in [-2047, 2047]
    buck = _rel_bucket(d)  # [4095]
    vals = rel_emb[buck, :].astype(np.float32)  # [4095, H]
    idx = np.arange(KC)[:, None] + (TBL - 1) - np.arange(TBL)[None, :]
    t = np.exp(vals[idx, :])  # [128, TBL, H]
    return np.ascontiguousarray(np.transpose(t, (2, 0, 1)))


# ------------------------------------------------------------- kernel body
def mha_body(tc, outs, ins, ckpt=None):
    nc = tc.nc
    ctx = ExitStack()
    xt_d = ins["xt"].bitcast(F32R)        # [1024, 2048]
    wq_d = ins["wq"].bitcast(F32R)        # [1024, 256]
    wk_d = ins["wk"].bitcast(F32R)        # [1024, 256]
    wv_d = ins["wv"].bitcast(F32R)        # [1024, 256]
    wo_d = ins["wo"].bitcast(F32R)        # [256, 1024]
    eb_d = ins["expb"]      # [HPC, 128, TBL] ATT_DT
    out_d = outs["out"]     # [2048, 1024] f32

    att_np = ATT_DT
    DKN = D_MODEL // 128    # 8 contraction chunks
    NQ = S // QB            # 4 q blocks
    NK = S // KC            # 16 k chunks

    def r(ap):  # operands already float32r
        return ap

    with ctx:
        const = ctx.enter_context(tc.tile_pool(name="const", bufs=1))

        # ---- persistent SBUF tensors
        qt = [const.tile([128, S], F32R, tag=f"qt{i}", name=f"qt{i}") for i in range(2)]
        kt = [const.tile([128, S], F32R, tag=f"kt{i}", name=f"kt{i}") for i in range(2)]
        # V with a ones column per head: [k, 4*65]; bf16 (AV stationary)
        vsb = [const.tile([128, HPC * 65], att_np, tag=f"v{i}", name=f"v{i}") for i in range(NK)]
        # normalized attention outputs, head-pairs stacked on partitions
        ust = [const.tile([128, S], F32R, tag=f"ust{i}", name=f"ust{i}") for i in range(2)]
        wo = [const.tile([128, D_MODEL], F32R, tag=f"wo{i}", name=f"wo{i}") for i in range(2)]
        ebp = ctx.enter_context(tc.tile_pool(name="expb", bufs=2))

        for i in range(2):
            nc.sync.dma_start(out=wo[i], in_=wo_d[i * 128:(i + 1) * 128, :])
        # prefetch head-pair 0's exp-bias tables before phase 1 so the
        # attention pipeline never stalls the PE at the phase boundary
        ebs = {}
        for j in range(2):
            eb = ebp.tile([128, TBL], att_np, tag=f"eb{j}", name=f"eb0_{j}")
            nc.sync.dma_start(out=eb, in_=eb_d[j])
            ebs[(0, j)] = eb

        # ---- flat pools (no scoped release: pool-release barriers idle the
        # PE >3.4us at phase boundaries and drop the HAM clock to 1.2GHz)
        wpool = ctx.enter_context(tc.tile_pool(name="wqkv", bufs=1))
        xtp = ctx.enter_context(tc.tile_pool(name="xts", bufs=6))
        xtvp = ctx.enter_context(tc.tile_pool(name="xtv", bufs=6))
        esp = ctx.enter_context(tc.tile_pool(name="es", bufs=4))
        esbp = ctx.enter_context(tc.tile_pool(name="esb", bufs=4))
        rzp = ctx.enter_context(tc.tile_pool(name="rz", bufs=2))
        outp = ctx.enter_context(tc.tile_pool(name="outsb", bufs=3))
        # one PSUM pool, 4 tags x 2 bufs of [128,512]f32 = 8 banks, shared by
        # all phases (no psum pool release barriers)
        pp = ctx.enter_context(tc.tile_pool(name="pp", bufs=2, space="PSUM"))

        # ================= phase 1: projections =================
        wq = [wpool.tile([128, DH], F32R, tag=f"wq{i}", name=f"wq{i}") for i in range(DKN)]
        wk = [wpool.tile([128, DH], F32R, tag=f"wk{i}", name=f"wk{i}") for i in range(DKN)]
        wv = [wpool.tile([128, DH], F32R, tag=f"wv{i}", name=f"wv{i}") for i in range(DKN)]
        for i in range(DKN):
            nc.sync.dma_start(out=wq[i], in_=wq_d[i * 128:(i + 1) * 128, :])
            nc.sync.dma_start(out=wk[i], in_=wk_d[i * 128:(i + 1) * 128, :])
            nc.sync.dma_start(out=wv[i], in_=wv_d[i * 128:(i + 1) * 128, :])

        # QK pass: xT streamed once; all 4 projections accumulate per block
        for qb in range(NQ):
            pq = [pp.tile([128, QB], F32, tag=t, name=f"pq{m}_{qb}")
                  for m, t in ((0, "a"), (1, "b"))]
            pk = [pp.tile([128, QB], F32, tag=t, name=f"pk{m}_{qb}")
                  for m, t in ((0, "c"), (1, "d"))]
            for dk in range(DKN):
                xtt = xtp.tile([128, QB], F32R, tag="xts", name=f"xt_{qb}_{dk}")
                nc.sync.dma_start(
                    out=xtt,
                    in_=xt_d[dk * 128:(dk + 1) * 128, qb * QB:(qb + 1) * QB])
                for m in range(2):
                    nc.tensor.matmul(
                        pq[m], wq[dk][:, m * 128:(m + 1) * 128], xtt,
                        start=(dk == 0), stop=(dk == DKN - 1))
                    nc.tensor.matmul(
                        pk[m], wk[dk][:, m * 128:(m + 1) * 128], xtt,
                        start=(dk == 0), stop=(dk == DKN - 1))
            for m in range(2):
                nc.scalar.copy(out=qt[m][:, qb * QB:(qb + 1) * QB], in_=pq[m])
                nc.scalar.copy(out=kt[m][:, qb * QB:(qb + 1) * QB], in_=pk[m])

        # V pass: xT re-streamed as [128,128] stationary tiles
        for kc in range(NK):
            pv = pp.tile([128, DH], F32, tag="abcd"[kc % 4], name=f"pv{kc}")
            for dk in range(DKN):
                xtt = xtvp.tile([128, 128], F32R, tag="xtv",
                                name=f"xtv_{kc}_{dk}")
                nc.sync.dma_start(
                    out=xtt,
                    in_=xt_d[dk * 128:(dk + 1) * 128, kc * 128:(kc + 1) * 128])
                nc.tensor.matmul(pv, xtt, wv[dk],
                                 start=(dk == 0), stop=(dk == DKN - 1))
            v3 = vsb[kc].rearrange("p (h c) -> p h c", h=HPC)
            nc.scalar.copy(
                out=v3[:, :, 0:64],
                in_=pv.rearrange("p (h c) -> p h c", h=HPC))
            nc.vector.memset(v3[:, :, 64:65], 1.0)

        # ================= phase 2+3: attention =================
        # Heads in pairs (2hp, 2hp+1): score matmuls use disjoint PE row
        # groups (rows 0-63 / 64-127) so weight loads + streaming overlap.
        mulc = 0  # round-robin DVE/GpSimd mult offload
        for hp in range(2):
            if hp == 0:
                # prefetch pair 1's tables during pair 0's compute
                for j in range(2):
                    eb = ebp.tile([128, TBL], att_np, tag=f"eb{j}",
                                  name=f"eb1_{j}")
                    nc.sync.dma_start(out=eb, in_=eb_d[2 + j])
                    ebs[(1, j)] = eb
            for qb in range(NQ):
                pus = [pp.tile([65, QB], F32, tag=t, name=f"pu{j}_{hp}_{qb}")
                       for j, t in ((0, "c"), (1, "d"))]
                for kc in range(NK):
                    base = (TBL - S) - kc * 128 + qb * QB
                    pss, ess, esbs = [], [], []
                    for j in range(2):
                        prow = slice(j * 64, j * 64 + 64)
                        ps = pp.tile([128, QB], F32, tag="ab"[j],
                                     name=f"ps{j}_{kc}")
                        nc.tensor.matmul(
                            ps,
                            kt[hp][prow, kc * 128:(kc + 1) * 128],
                            qt[hp][prow, qb * QB:(qb + 1) * QB],
                            start=True, stop=True)
                        pss.append(ps)
                    for j in range(2):
                        es = esp.tile([128, QB], att_np, tag=f"es{j}",
                                      name=f"es{j}_{kc}")
                        nc.scalar.activation(out=es, in_=pss[j], func=AF.Exp)
                        ess.append(es)
                    for j in range(2):
                        esb = esbp.tile([128, QB], att_np, tag=f"esb{j}",
                                        name=f"esb{j}_{kc}")
                        eng = nc.gpsimd if (mulc % 3 == 2) else nc.vector
                        eng.tensor_mul(esb, ess[j],
                                       ebs[(hp, j)][:, base:base + QB])
                        mulc += 1
                        esbs.append(esb)
                    for j in range(2):
                        h = hp * 2 + j
                        nc.tensor.matmul(
                            pus[j], vsb[kc][:, h * 65:(h + 1) * 65], esbs[j],
                            start=(kc == 0), stop=(kc == NK - 1))
                # normalize U[d, q] / Z[q]; Z = row 64 of pu
                for j in range(2):
                    rz = rzp.tile([1, QB], F32, tag=f"rz{j}", name=f"rz{j}_{qb}")
                    nc.vector.reciprocal(out=rz, in_=pus[j][64:65, :])
                    rzb = rzp.tile([64, QB], F32, tag=f"rzb{j}",
                                   name=f"rzb{j}_{qb}")
                    nc.gpsimd.partition_broadcast(rzb, rz, channels=64)
                    if j == 0:
                        nc.vector.tensor_mul(
                            ust[hp][0:64, qb * QB:(qb + 1) * QB],
                            pus[j][0:64, :], rzb)
                    else:
                        # DVE lanes are partition-locked; write via a [64,512]
                        # staging tile then DMA to rows 64-127
                        stg = rzp.tile([64, QB], F32R, tag="stg",
                                       name=f"stg{hp}_{qb}")
                        nc.vector.tensor_mul(stg, pus[j][0:64, :], rzb)
                        nc.sync.dma_start(
                            out=ust[hp][64:128, qb * QB:(qb + 1) * QB],
                            in_=stg)

        # ================= phase 4: output projection =================
        for qc in range(S // 128):
            ob = outp.tile([128, D_MODEL], F32, tag="ob", name=f"ob{qc}")
            for e in range(2):
                po = pp.tile([128, 512], F32, tag="ab"[e], name=f"po{qc}_{e}")
                for i in range(2):
                    nc.tensor.matmul(
                        po,
                        ust[i][:, qc * 128:(qc + 1) * 128],
                        wo[i][:, e * 512:(e + 1) * 512],
                        start=(i == 0), stop=(i == 1))
                nc.vector.tensor_copy(out=ob[:, e * 512:(e + 1) * 512], in_=po)
            nc.sync.dma_start(out=out_d[qc * 128:(qc + 1) * 128, :], in_=ob)


# ------------------------------------------------------------- build + run
def _build():
    if "nc" in _cache:
        return _cache["nc"]
    nc = bacc.Bacc("TRN2", target_bir_lowering=False, debug=False)
    att_np_dt = mybir.dt.np(ATT_DT)
    ins = {
        "xt": nc.dram_tensor("xt", [D_MODEL, S], F32R, kind="ExternalInput").ap(),
        "wq": nc.dram_tensor("wq", [D_MODEL, DH], F32R, kind="ExternalInput").ap(),
        "wk": nc.dram_tensor("wk", [D_MODEL, DH], F32R, kind="ExternalInput").ap(),
        "wv": nc.dram_tensor("wv", [D_MODEL, DH], F32R, kind="ExternalInput").ap(),
        "wo": nc.dram_tensor("wo", [DH, D_MODEL], F32R, kind="ExternalInput").ap(),
        "expb": nc.dram_tensor("expb", [HPC, KC, TBL], ATT_DT,
                               kind="ExternalInput").ap(),
    }
    outs = {
        "out": nc.dram_tensor("out", [S, D_MODEL], F32, kind="ExternalOutput").ap(),
    }
    with tile.TileContext(nc) as tc:
        mha_body(tc, outs, ins)
    nc.compile()
    _cache["nc"] = nc
    return nc


TRACE = False
LAST = {}


def kernel(inputs, Wq, Wk, Wv, Wo, rel_emb):
    inputs = np.asarray(inputs, dtype=np.float32)
    Wq = np.asarray(Wq, dtype=np.float32)
    Wk = np.asarray(Wk, dtype=np.float32)
    Wv = np.asarray(Wv, dtype=np.float32)
    Wo = np.asarray(Wo, dtype=np.float32)
    rel_emb = np.asarray(rel_emb, dtype=np.float32)

    nc = _build()
    att_np_dt = mybir.dt.np(ATT_DT)

    ebt = _expbias_tables(rel_emb)  # [16, 128, TBL] f32
    in_maps = []
    for c in range(N_CORES):
        b, g = c // (N_CORES // B), c % (N_CORES // B)
        hs = slice(g * DH, (g + 1) * DH)
        in_maps.append({
            "xt": np.ascontiguousarray(inputs[b].T),
            "wq": np.ascontiguousarray(Wq[:, hs]),
            "wk": np.ascontiguousarray(Wk[:, hs]),
            "wv": np.ascontiguousarray(Wv[:, hs]),
            "wo": np.ascontiguousarray(Wo[hs, :]),
            "expb": np.ascontiguousarray(
                ebt[g * HPC:(g + 1) * HPC]).astype(att_np_dt),
        })

    res = run_bass_kernel_spmd(
        nc, in_maps, core_ids=list(range(N_CORES)), trace=TRACE)
    LAST["res"] = res

    out = np.zeros((B, S, D_MODEL), dtype=np.float64)
    for c in range(N_CORES):
        b = c // (N_CORES // B)
        out[b] += res.results[c]["out"].astype(np.float64)
    return out.astype(np.float32)



# revision 35
# speedup vs baseline: 130.6180x; 130.6180x over previous
"""T5-style MultiHeadAttention (relative position bias) on 8 Trainium2 cores.

Sharding: core c = (b, g) with b = c // 4 (batch), g = c % 4 (head group of 4
heads).  Each core computes q/k/v projections for its 4 heads, attention with
the relative-position bias, and a partial output projection (rows of Wo for
its heads).  Host sums the 4 partials per batch element.

Key layout choices (per core):
  - x is passed transposed: xT [1024, 2048] so projections contract over
    partitions directly.  xT is streamed ONCE: the V projection is fused into
    the QK pass (V's stationary [128,128] chunks are slices of the same
    [128,4096] xT tile), so phase 1 does 4 big 2MB DMAs instead of 160 small
    ones (the kernel is DMA-bound: measured marginal HBM cost ~8.5ns/KB).
  - Wq|Wk|Wv are concatenated host-side into one [1024, 768] tensor loaded
    with a single 3MB DMA; Wo's two row-halves load with one 1MB DMA.
  - Q_t, K_t stored as [d, seq] (d on partitions); scores computed
    *transposed* as S_t[k, q] so exp(S_t) feeds the attn@V matmul directly
    (contracting over k = partitions).
  - Softmax denominator Z[q] = sum_k exp(S_t[k, q]) falls out of the attn@V
    matmul for free via a ones-column appended to V (M=65 per head).
  - No max-subtraction: scores are O(50) at most, exp stays finite.
  - Score PSUM tiles are [128, 1024] (2 banks, kc pairs) so exp runs at
    FD=1024, amortizing the ACT 352-cycle instruction overhead.
  - Relative-position bias applied multiplicatively after exp:
    exp(s + bias) = exp(s) * exp(bias).  bias[k, q] = v_h[k - q + 2047] is
    Toeplitz, so exp(bias) tiles are overlapping windows of a single
    [128, 3968] sliding table per head, precomputed on host:
      T_h[p, i] = exp(v_h[p + 3967 - i])
    and the tile for (k0 = kc*128, q0 = qb*512) is T_h[:, base:base+512] with
    base = 1920 - k0 + q0.
  - Matmuls run in float32r (full PE rate at N>=256); the attention
    probabilities / V use bf16 for the 2x DVE multiply mode.
"""

import numpy as np
from contextlib import ExitStack

import concourse.bass as bass
import concourse.tile as tile
from concourse import bacc, mybir
from concourse.bass_utils import run_bass_kernel_spmd

# ---------------------------------------------------------------- constants
B, S, D_MODEL, N_HEADS, D_KV = 2, 2048, 1024, 16, 64
NUM_BUCKETS, MAX_DIST = 32, 128
N_CORES = 8
HPC = N_HEADS // (N_CORES // B)  # heads per core = 4
DH = HPC * D_KV                  # 256 d-cols per core
TBL = 3968                       # exp-bias sliding table width
QB = 512                         # q block (free dim of score tiles)
KC = 128                         # k chunk (partition dim of score tiles)
DKN = D_MODEL // 128             # 8 contraction chunks
NQ = S // QB                     # 4 q blocks
NK = S // KC                     # 16 k chunks

F32 = mybir.dt.float32
F32R = mybir.dt.float32r
BF16 = mybir.dt.bfloat16
AF = mybir.ActivationFunctionType

# attention-probability dtype: BF16 (fast DVE 2x) or F32 (accurate, 1x DVE)
ATT_DT = BF16

# bottleneck probes (bench-only; all False for the graded kernel): duplicate
# a resource's work to measure its marginal wall cost under the rep-loop bench
PROBE_DMA2 = False    # re-issue every xt tile DMA (adds ~8MB HBM traffic)
PROBE_ACT2 = False    # re-issue every exp activation into a scratch tile
PROBE_NORM2 = False   # re-issue the normalize tail (recip+bcast+mul) to scratch
PROBE_P1X2 = False    # emit phase 1 (projections) twice
PROBE_P4X2 = False    # emit phase 4 (output projection) twice
PROBE_MUL2 = False    # double-write every es*expbias DVE multiply
PROBE_PE2 = False     # double-write every score matmul
PROBE_PHASES = "all"  # "p1": phase1+dummy-out; "p12": +attention; "all"
DEBUG_DUMP = False    # export qt/kt/vsb/ust intermediates for phase triage

_cache = {}


# ------------------------------------------------------------- host helpers
def _rel_bucket(d):
    """Bucket of relative position d = k - q (bidirectional T5), numpy fp32
    mirror of the jax reference."""
    nb = NUM_BUCKETS // 2
    n = -d
    ret = (n < 0).astype(np.int32) * nb
    n = np.abs(n)
    max_exact = nb // 2
    is_small = n < max_exact
    nf = np.maximum(n, 1).astype(np.float32)
    val = (
        np.log(nf / np.float32(max_exact))
        / np.float32(np.log(MAX_DIST / max_exact))
        * np.float32(nb - max_exact)
    ).astype(np.int32) + max_exact
    val = np.minimum(val, nb - 1)
    return ret + np.where(is_small, n, val)


def _expbias_tables(rel_emb):
    """[N_HEADS, 128, TBL] exp-bias sliding tables (float32)."""
    d = np.arange(-(S - 1), S)  # k - q in [-2047, 2047]
    buck = _rel_bucket(d)  # [4095]
    vals = rel_emb[buck, :].astype(np.float32)  # [4095, H]
    idx = np.arange(KC)[:, None] + (TBL - 1) - np.arange(TBL)[None, :]
    t = np.exp(vals[idx, :])  # [128, TBL, H]
    return np.ascontiguousarray(np.transpose(t, (2, 0, 1)))


def make_in_maps(inputs, Wq, Wk, Wv, Wo, rel_emb):
    """Per-core input dict list (shared by kernel(), test.py, bench_hw.py)."""
    att_np_dt = mybir.dt.np(ATT_DT)
    ebt = _expbias_tables(np.asarray(rel_emb, np.float32))
    in_maps = []
    for c in range(N_CORES):
        b, g = c // (N_CORES // B), c % (N_CORES // B)
        hs = slice(g * DH, (g + 1) * DH)
        wqkv = np.concatenate(
            [np.asarray(Wq, np.float32)[:, hs],
             np.asarray(Wk, np.float32)[:, hs],
             np.asarray(Wv, np.float32)[:, hs]], axis=1)  # [1024, 768]
        in_maps.append({
            "xt": np.ascontiguousarray(np.asarray(inputs, np.float32)[b].T),
            "wqkv": np.ascontiguousarray(wqkv),
            "wo": np.ascontiguousarray(np.asarray(Wo, np.float32)[hs, :]),
            "expb": np.ascontiguousarray(
                ebt[g * HPC:(g + 1) * HPC]).astype(att_np_dt),
        })
    return in_maps


# ------------------------------------------------------------- kernel body
def mha_body(tc, outs, ins, reps=1):
    """reps>1 wraps the whole compute in a hardware loop (For_i) — used only
    by the benchmark to measure per-iteration HW time with the fixed axon
    dispatch overhead differenced out."""
    nc = tc.nc
    ctx = ExitStack()
    att_np = ATT_DT

    with ctx:
        const = ctx.enter_context(tc.tile_pool(name="const", bufs=1))

        # ---- persistent SBUF tensors
        qt = [const.tile([128, S], F32R, tag=f"qt{i}", name=f"qt{i}")
              for i in range(2)]
        kt = [const.tile([128, S], F32R, tag=f"kt{i}", name=f"kt{i}")
              for i in range(2)]
        # V with a ones column per head: [k, 4*65]; bf16 (AV stationary)
        vsb = [const.tile([128, HPC * 65], att_np, tag=f"v{i}", name=f"v{i}")
               for i in range(NK)]
        # normalized attention outputs, head-pairs stacked on partitions
        ust = [const.tile([128, S], F32R, tag=f"ust{i}", name=f"ust{i}")
               for i in range(2)]
        # merged weights: wt[:, dk*768 + {q:0,k:256,v:512} + c]
        wt = const.tile([128, DKN * 3 * DH], F32R, tag="wt", name="wt")
        # merged Wo: wot[:, i*1024 + c] for row-half i
        wot = const.tile([128, 2 * D_MODEL], F32R, tag="wot", name="wot")

        ebp = ctx.enter_context(tc.tile_pool(name="expb", bufs=2))
        xtqp = ctx.enter_context(tc.tile_pool(name="xtq", bufs=2))
        esp = ctx.enter_context(tc.tile_pool(name="es", bufs=2))
        esbp = ctx.enter_context(tc.tile_pool(name="esb", bufs=4))
        rzp = ctx.enter_context(tc.tile_pool(name="rz", bufs=2))
        outp = ctx.enter_context(tc.tile_pool(name="outsb", bufs=2))
        # PSUM: pps = 2 tags x 1 buf of [128,1024] (4 banks, scores/proj),
        #       ppu = 2 tags x 2 bufs of [128,512] (4 banks, AV accum/V/out)
        pps = ctx.enter_context(tc.tile_pool(name="pps", bufs=1, space="PSUM"))
        ppu = ctx.enter_context(tc.tile_pool(name="ppu", bufs=2, space="PSUM"))

        ns = dict(const=const, ebp=ebp, xtqp=xtqp, esp=esp, esbp=esbp,
                  rzp=rzp, outp=outp, pps=pps, ppu=ppu, qt=qt, kt=kt,
                  vsb=vsb, ust=ust, wt=wt, wot=wot, ins=ins, outs=outs,
                  att_np=att_np)

        if reps == 1:
            _emit_body(tc, nc, ns)
        else:
            with tc.For_i(0, reps):
                _emit_body(tc, nc, ns)


def _emit_body(tc, nc, ns):
    (const, ebp, xtqp, esp, esbp, rzp, outp, pps, ppu, qt, kt, vsb, ust, wt,
     wot, ins, outs, att_np) = (ns[k] for k in (
        "const", "ebp", "xtqp", "esp", "esbp", "rzp", "outp", "pps", "ppu",
        "qt", "kt", "vsb", "ust", "wt", "wot", "ins", "outs", "att_np"))

    xt_d = ins["xt"].bitcast(F32R)        # [1024, 2048]
    wqkv_d = ins["wqkv"].bitcast(F32R)    # [1024, 768]
    wo_d = ins["wo"].bitcast(F32R)        # [256, 1024]
    eb_d = ins["expb"]                    # [HPC, 128, TBL] ATT_DT
    out_d = outs["out"]                   # [2048, 1024] f32

    xtr = xt_d.rearrange("(dk p) q -> p dk q", dk=DKN)     # [128, 8, 2048]

    # ---- weight + bias-table loads (one 3MB, one 1MB, 4x 1MB DMAs)
    nc.sync.dma_start(
        out=wt.rearrange("p (dk c) -> p dk c", dk=DKN),
        in_=wqkv_d.rearrange("(dk p) c -> p dk c", dk=DKN))
    nc.sync.dma_start(
        out=wot.rearrange("p (i c) -> p i c", i=2),
        in_=wo_d.rearrange("(i p) c -> p i c", i=2))
    ebs = {}
    for j in range(2):
        eb = ebp.tile([128, TBL], att_np, tag=f"eb{j}", name=f"eb0_{j}")
        nc.sync.dma_start(out=eb, in_=eb_d[j])
        ebs[(0, j)] = eb

    def wq_s(dk):
        return wt[:, dk * 768:dk * 768 + 256]

    def wk_s(dk):
        return wt[:, dk * 768 + 256:dk * 768 + 512]

    def wv_s(dk):
        return wt[:, dk * 768 + 512:dk * 768 + 768]

    # ================= phase 1: fused QKV projections =================
    # xT streamed once as [128, 4096] tiles; V's stationary chunks are
    # slices of the same tile (kc = 4*qb + t covers this q block).
    for _p1rep in range(2 if PROBE_P1X2 else 1):
      for qb in range(NQ):
        xtq = xtqp.tile([128, DKN * QB], F32R, tag="xtq", name=f"xtq{qb}")
        nc.sync.dma_start(
            out=xtq.rearrange("p (dk q) -> p dk q", dk=DKN),
            in_=xtr[:, :, qb * QB:(qb + 1) * QB])
        if PROBE_DMA2:
            nc.sync.dma_start(
                out=xtq.rearrange("p (dk q) -> p dk q", dk=DKN),
                in_=xtr[:, :, qb * QB:(qb + 1) * QB])

        pqm = [pps.tile([128, QB], F32, tag=f"s{m}", name=f"pq{m}_{qb}")
               for m in range(2)]
        pkm = [pps.tile([128, QB], F32, tag=f"s{m}", name=f"pk{m}_{qb}")
               for m in range(2)]
        pv2 = [ppu.tile([128, QB], F32, tag=f"u{i}", name=f"pv2_{qb}_{i}")
               for i in range(2)]
        for dk in range(DKN):
            xs = xtq[:, dk * QB:(dk + 1) * QB]
            first, last = dk == 0, dk == DKN - 1
            for m in range(2):
                nc.tensor.matmul(
                    pqm[m], wq_s(dk)[:, m * 128:(m + 1) * 128], xs,
                    start=first, stop=last)
                nc.tensor.matmul(
                    pkm[m], wk_s(dk)[:, m * 128:(m + 1) * 128], xs,
                    start=first, stop=last)
            for t in range(4):
                pv = pv2[t // 2][:, (t % 2) * DH:(t % 2 + 1) * DH]
                # start only on the bank's FIRST chunk: start=True clears
                # has_written for the WHOLE bank, so the t%2==1 chunk must
                # not re-clear (it would wipe t%2==0's dk=0 partials); its
                # dk=0 write lands as overwrite via cleared has_written bits.
                nc.tensor.matmul(pv, xs[:, t * 128:(t + 1) * 128], wv_s(dk),
                                 start=(first and t % 2 == 0), stop=last)
        # evacuate: Q/K on DVE, V on ACT (both have slack in phase 1)
        for m in range(2):
            nc.vector.tensor_copy(
                out=qt[m][:, qb * QB:(qb + 1) * QB], in_=pqm[m])
            nc.vector.tensor_copy(
                out=kt[m][:, qb * QB:(qb + 1) * QB], in_=pkm[m])
        for t in range(4):
            kc = 4 * qb + t
            v3 = vsb[kc].rearrange("p (h c) -> p h c", h=HPC)
            pv = pv2[t // 2][:, (t % 2) * DH:(t % 2 + 1) * DH]
            nc.scalar.copy(
                out=v3[:, :, 0:64],
                in_=pv.rearrange("p (h c) -> p h c", h=HPC))
            nc.vector.memset(v3[:, :, 64:65], 1.0)

    # ================= phase 2+3: attention =================
    # Heads in pairs (2hp, 2hp+1): score matmuls use disjoint PE row
    # groups (rows 0-63 / 64-127).  Score tiles are single-bank [128,512]
    # in 4 PSUM slots (2 tags x 2 bufs) so scores(kc+1) overlaps exp(kc);
    # the attn@V matmuls are emitted ONE kc LATE so the in-order PE queue
    # never stalls waiting for the exp->mul chain of the current kc.
    for hp in range(2 if PROBE_PHASES != "p1" else 0):
        if hp == 0:
            # prefetch pair 1's tables during pair 0's compute
            for j in range(2):
                eb = ebp.tile([128, TBL], att_np, tag=f"eb{j}",
                              name=f"eb1_{j}")
                nc.sync.dma_start(out=eb, in_=eb_d[2 + j])
                ebs[(1, j)] = eb
        for qb in range(NQ):
            pus = [ppu.tile([128, QB], F32, tag=f"u{j}",
                            name=f"pu{j}_{hp}_{qb}") for j in range(2)]
            esb_prev = None
            for kc in range(NK + 1):
                if kc < NK:
                    base = (TBL - S) - kc * 128 + qb * QB
                    sjt = [pps.tile([128, QB], F32, tag=f"s{j}",
                                    name=f"s{j}_{hp}_{qb}_{kc}")
                           for j in range(2)]
                    for j in range(2):
                        prow = slice(j * 64, j * 64 + 64)
                        for _r in range(2 if PROBE_PE2 else 1):
                            nc.tensor.matmul(
                                sjt[j],
                                kt[hp][prow, kc * 128:(kc + 1) * 128],
                                qt[hp][prow, qb * QB:(qb + 1) * QB],
                                start=True, stop=True)
                    esb_cur = []
                    for j in range(2):
                        es = esp.tile([128, QB], att_np, tag=f"es{j}",
                                      name=f"es{j}_{hp}_{qb}_{kc}")
                        nc.scalar.activation(out=es, in_=sjt[j], func=AF.Exp)
                        if PROBE_ACT2:
                            # SBUF-neutral: double-write the same tile (WAW)
                            nc.scalar.activation(out=es, in_=sjt[j],
                                                 func=AF.Exp)
                        esb = esbp.tile([128, QB], att_np, tag=f"esb{j}",
                                        name=f"esb{j}_{hp}_{qb}_{kc}")
                        for _r in range(2 if PROBE_MUL2 else 1):
                            nc.vector.tensor_mul(
                                esb, es, ebs[(hp, j)][:, base:base + QB])
                        esb_cur.append(esb)
                if kc >= 1:
                    kcp = kc - 1
                    for j in range(2):
                        h = hp * 2 + j
                        nc.tensor.matmul(
                            pus[j][0:65, :],
                            vsb[kcp][:, h * 65:(h + 1) * 65], esb_prev[j],
                            start=(kcp == 0), stop=(kcp == NK - 1))
                if kc < NK:
                    esb_prev = esb_cur
            # normalize U[d, q] / Z[q]; Z = row 64 of pu
            for j in range(2):
                rz = rzp.tile([1, QB], F32, tag=f"rz{j}", name=f"rz{j}_{qb}")
                nc.vector.reciprocal(out=rz, in_=pus[j][64:65, :])
                rzb = rzp.tile([64, QB], F32, tag=f"rzb{j}",
                               name=f"rzb{j}_{qb}")
                nc.gpsimd.partition_broadcast(rzb, rz, channels=64)
                if PROBE_NORM2:
                    # SBUF-neutral: double-write the same tiles (WAW)
                    nc.vector.reciprocal(out=rz, in_=pus[j][64:65, :])
                    nc.gpsimd.partition_broadcast(rzb, rz, channels=64)
                if j == 0:
                    nc.vector.tensor_mul(
                        ust[hp][0:64, qb * QB:(qb + 1) * QB],
                        pus[j][0:64, :], rzb)
                else:
                    # DVE lanes are partition-locked; write via a [64,512]
                    # staging tile then DMA to rows 64-127
                    stg = rzp.tile([64, QB], F32R, tag="stg",
                                   name=f"stg{hp}_{qb}")
                    nc.vector.tensor_mul(stg, pus[j][0:64, :], rzb)
                    nc.sync.dma_start(
                        out=ust[hp][64:128, qb * QB:(qb + 1) * QB],
                        in_=stg)

    # ================= phase 4: output projection =================
    # two 128-row chunks per [128,2048] staging tile -> 8x 1MB out DMAs
    if PROBE_PHASES != "all":
        # timing-only variants: write a dummy output (no ust dependency)
        for qg in range(S // 256):
            ob = outp.tile([128, 2 * D_MODEL], F32, tag="ob", name=f"ob{qg}")
            nc.vector.memset(ob, 0.0)
            nc.sync.dma_start(
                out=out_d[qg * 256:(qg + 1) * 256, :].rearrange(
                    "(h p) c -> p h c", h=2),
                in_=ob.rearrange("p (h c) -> p h c", h=2))
        return
    for _p4rep in range(2 if PROBE_P4X2 else 1):
      for qg in range(S // 256):
        ob = outp.tile([128, 2 * D_MODEL], F32, tag="ob", name=f"ob{qg}")
        for half in range(2):
            qc = 2 * qg + half
            for e in range(2):
                po = ppu.tile([128, QB], F32, tag=f"u{e}",
                              name=f"po{qc}_{e}")
                for i in range(2):
                    nc.tensor.matmul(
                        po,
                        ust[i][:, qc * 128:(qc + 1) * 128],
                        wot[:, i * D_MODEL + e * QB:i * D_MODEL + (e + 1) * QB],
                        start=(i == 0), stop=(i == 1))
                nc.vector.tensor_copy(
                    out=ob[:, half * D_MODEL + e * QB:half * D_MODEL + (e + 1) * QB],
                    in_=po)
        nc.sync.dma_start(
            out=out_d[qg * 256:(qg + 1) * 256, :].rearrange(
                "(h p) c -> p h c", h=2),
            in_=ob.rearrange("p (h c) -> p h c", h=2))

    if DEBUG_DUMP:
        nc.sync.dma_start(out=outs["dbg_qt0"].bitcast(F32R), in_=qt[0])
        nc.sync.dma_start(out=outs["dbg_kt0"].bitcast(F32R), in_=kt[0])
        nc.sync.dma_start(out=outs["dbg_v0"], in_=vsb[0])
        nc.sync.dma_start(out=outs["dbg_ust0"].bitcast(F32R), in_=ust[0])
        nc.sync.dma_start(out=outs["dbg_wt"].bitcast(F32R), in_=wt)


# ------------------------------------------------------------- build + run
def _build(reps=1):
    key = ("nc", reps, PROBE_DMA2, PROBE_ACT2, PROBE_NORM2, PROBE_P1X2,
           PROBE_P4X2, PROBE_MUL2, PROBE_PE2, PROBE_PHASES, DEBUG_DUMP)
    if key in _cache:
        return _cache[key]
    nc = bacc.Bacc("TRN2", target_bir_lowering=False, debug=False)
    ins = {
        "xt": nc.dram_tensor("xt", [D_MODEL, S], F32R, kind="ExternalInput").ap(),
        "wqkv": nc.dram_tensor("wqkv", [D_MODEL, 3 * DH], F32R,
                               kind="ExternalInput").ap(),
        "wo": nc.dram_tensor("wo", [DH, D_MODEL], F32R, kind="ExternalInput").ap(),
        "expb": nc.dram_tensor("expb", [HPC, KC, TBL], ATT_DT,
                               kind="ExternalInput").ap(),
    }
    outs = {
        "out": nc.dram_tensor("out", [S, D_MODEL], F32, kind="ExternalOutput").ap(),
    }
    if DEBUG_DUMP:
        outs["dbg_qt0"] = nc.dram_tensor(
            "dbg_qt0", [128, S], F32, kind="ExternalOutput").ap()
        outs["dbg_kt0"] = nc.dram_tensor(
            "dbg_kt0", [128, S], F32, kind="ExternalOutput").ap()
        outs["dbg_v0"] = nc.dram_tensor(
            "dbg_v0", [128, HPC * 65], ATT_DT, kind="ExternalOutput").ap()
        outs["dbg_ust0"] = nc.dram_tensor(
            "dbg_ust0", [128, S], F32, kind="ExternalOutput").ap()
        outs["dbg_wt"] = nc.dram_tensor(
            "dbg_wt", [128, DKN * 3 * DH], F32, kind="ExternalOutput").ap()
    with tile.TileContext(nc) as tc:
        mha_body(tc, outs, ins, reps=reps)
    nc.compile()
    _cache[key] = nc
    return nc


TRACE = False
LAST = {}


def kernel(inputs, Wq, Wk, Wv, Wo, rel_emb):
    nc = _build()
    in_maps = make_in_maps(inputs, Wq, Wk, Wv, Wo, rel_emb)

    res = run_bass_kernel_spmd(
        nc, in_maps, core_ids=list(range(N_CORES)), trace=TRACE)
    LAST["res"] = res

    out = np.zeros((B, S, D_MODEL), dtype=np.float64)
    for c in range(N_CORES):
        b = c // (N_CORES // B)
        out[b] += res.results[c]["out"].astype(np.float64)
    return out.astype(np.float32)


# revision 42
# speedup vs baseline: 143.4252x; 1.0981x over previous
"""T5-style MultiHeadAttention (relative position bias) on 8 Trainium2 cores.

Sharding: core c = (b, g) with b = c // 4 (batch), g = c % 4 (head group of 4
heads).  Each core computes q/k/v projections for its 4 heads, attention with
the relative-position bias, and a partial output projection (rows of Wo for
its heads).  Host sums the 4 partials per batch element.

Key layout choices (per core):
  - x is passed transposed: xT [1024, 2048] so projections contract over
    partitions directly.  xT is streamed ONCE: the V projection is fused into
    the QK pass (V's stationary [128,128] chunks are slices of the same
    [128,4096] xT tile), so phase 1 does 4 big 2MB DMAs instead of 160 small
    ones (the kernel is DMA-bound: measured marginal HBM cost ~8.5ns/KB).
  - Wq|Wk|Wv are concatenated host-side into one [1024, 768] tensor loaded
    with a single 3MB DMA; Wo's two row-halves load with one 1MB DMA.
  - Q_t, K_t stored as [d, seq] (d on partitions); scores computed
    *transposed* as S_t[k, q] so exp(S_t) feeds the attn@V matmul directly
    (contracting over k = partitions).
  - Softmax denominator Z[q] = sum_k exp(S_t[k, q]) falls out of the attn@V
    matmul for free via a ones-column appended to V (M=65 per head).
  - No max-subtraction: scores are O(50) at most, exp stays finite.
  - Score PSUM tiles are [128, 1024] (2 banks, kc pairs) so exp runs at
    FD=1024, amortizing the ACT 352-cycle instruction overhead.
  - Relative-position bias applied multiplicatively after exp:
    exp(s + bias) = exp(s) * exp(bias).  bias[k, q] = v_h[k - q + 2047] is
    Toeplitz, so exp(bias) tiles are overlapping windows of a single
    [128, 3968] sliding table per head, precomputed on host:
      T_h[p, i] = exp(v_h[p + 3967 - i])
    and the tile for (k0 = kc*128, q0 = qb*512) is T_h[:, base:base+512] with
    base = 1920 - k0 + q0.
  - Matmuls run in float32r (full PE rate at N>=256); the attention
    probabilities / V use bf16 for the 2x DVE multiply mode.
"""

import numpy as np
from contextlib import ExitStack

import concourse.bass as bass
import concourse.tile as tile
from concourse import bacc, mybir
from concourse.bass_utils import run_bass_kernel_spmd

# ---------------------------------------------------------------- constants
B, S, D_MODEL, N_HEADS, D_KV = 2, 2048, 1024, 16, 64
NUM_BUCKETS, MAX_DIST = 32, 128
N_CORES = 8
HPC = N_HEADS // (N_CORES // B)  # heads per core = 4
DH = HPC * D_KV                  # 256 d-cols per core
TBL = 3968                       # exp-bias sliding table width
QB = 512                         # q block (free dim of score tiles)
KC = 128                         # k chunk (partition dim of score tiles)
DKN = D_MODEL // 128             # 8 contraction chunks
NQ = S // QB                     # 4 q blocks
NK = S // KC                     # 16 k chunks

F32 = mybir.dt.float32
F32R = mybir.dt.float32r
BF16 = mybir.dt.bfloat16
AF = mybir.ActivationFunctionType

# attention-probability dtype: BF16 (fast DVE 2x) or F32 (accurate, 1x DVE)
ATT_DT = BF16

# bottleneck probes (bench-only; all False for the graded kernel): duplicate
# a resource's work to measure its marginal wall cost under the rep-loop bench
PROBE_DMA2 = False    # re-issue every xt tile DMA (adds ~8MB HBM traffic)
PROBE_ACT2 = False    # re-issue every exp activation into a scratch tile
PROBE_NORM2 = False   # re-issue the normalize tail (recip+bcast+mul) to scratch
PROBE_P1X2 = False    # emit phase 1 (projections) twice
PROBE_P4X2 = False    # emit phase 4 (output projection) twice
PROBE_MUL2 = False    # double-write every es*expbias DVE multiply
PROBE_PE2 = False     # double-write every score matmul
PROBE_PHASES = "all"  # "p1": phase1+dummy-out; "p12": +attention; "all"
DEBUG_DUMP = False    # export qt/kt/vsb/ust intermediates for phase triage

_cache = {}


# ------------------------------------------------------------- host helpers
def _rel_bucket(d):
    """Bucket of relative position d = k - q (bidirectional T5), numpy fp32
    mirror of the jax reference."""
    nb = NUM_BUCKETS // 2
    n = -d
    ret = (n < 0).astype(np.int32) * nb
    n = np.abs(n)
    max_exact = nb // 2
    is_small = n < max_exact
    nf = np.maximum(n, 1).astype(np.float32)
    val = (
        np.log(nf / np.float32(max_exact))
        / np.float32(np.log(MAX_DIST / max_exact))
        * np.float32(nb - max_exact)
    ).astype(np.int32) + max_exact
    val = np.minimum(val, nb - 1)
    return ret + np.where(is_small, n, val)


def _expbias_tables(rel_emb):
    """[N_HEADS, 128, TBL] exp-bias sliding tables (float32)."""
    d = np.arange(-(S - 1), S)  # k - q in [-2047, 2047]
    buck = _rel_bucket(d)  # [4095]
    vals = rel_emb[buck, :].astype(np.float32)  # [4095, H]
    idx = np.arange(KC)[:, None] + (TBL - 1) - np.arange(TBL)[None, :]
    t = np.exp(vals[idx, :])  # [128, TBL, H]
    return np.ascontiguousarray(np.transpose(t, (2, 0, 1)))


def _const_tile_bucket(kc, qb):
    """If the whole (kc, qb) score tile maps to ONE rel-pos bucket (the T5
    bucketing saturates for |k-q| >= 91), return it; else None.  Static
    geometry — data-independent."""
    d_lo = kc * KC - qb * QB - (QB - 1)
    d_hi = kc * KC + (KC - 1) - qb * QB
    u = np.unique(_rel_bucket(np.arange(d_lo, d_hi + 1)))
    return int(u[0]) if len(u) == 1 else None


def make_in_maps(inputs, Wq, Wk, Wv, Wo, rel_emb):
    """Per-core input dict list (shared by kernel(), test.py, bench_hw.py)."""
    att_np_dt = mybir.dt.np(ATT_DT)
    rel_emb = np.asarray(rel_emb, np.float32)
    ebt = _expbias_tables(rel_emb)
    in_maps = []
    for c in range(N_CORES):
        b, g = c // (N_CORES // B), c % (N_CORES // B)
        hs = slice(g * DH, (g + 1) * DH)
        wqkv = np.concatenate(
            [np.asarray(Wq, np.float32)[:, hs],
             np.asarray(Wk, np.float32)[:, hs],
             np.asarray(Wv, np.float32)[:, hs]], axis=1)  # [1024, 768]
        # saturated-bucket bias constants per local head: col h = bucket 15
        # (k<q side), col HPC+h = bucket 31 (k>q side); pre-broadcast to 128
        # partitions for use as an ACT per-partition bias operand.
        cvec = np.concatenate([rel_emb[15, g * HPC:(g + 1) * HPC],
                               rel_emb[31, g * HPC:(g + 1) * HPC]])
        in_maps.append({
            "xt": np.ascontiguousarray(np.asarray(inputs, np.float32)[b].T),
            "wqkv": np.ascontiguousarray(wqkv),
            "wo": np.ascontiguousarray(np.asarray(Wo, np.float32)[hs, :]),
            "expb": np.ascontiguousarray(
                ebt[g * HPC:(g + 1) * HPC]).astype(att_np_dt),
            "bcn": np.ascontiguousarray(
                np.broadcast_to(cvec, (128, 2 * HPC)).astype(np.float32)),
        })
    return in_maps


# ------------------------------------------------------------- kernel body
def mha_body(tc, outs, ins, reps=1):
    """reps>1 wraps the whole compute in a hardware loop (For_i) — used only
    by the benchmark to measure per-iteration HW time with the fixed axon
    dispatch overhead differenced out."""
    nc = tc.nc
    ctx = ExitStack()
    att_np = ATT_DT

    with ctx:
        const = ctx.enter_context(tc.tile_pool(name="const", bufs=1))

        # ---- persistent SBUF tensors
        qt = [const.tile([128, S], F32R, tag=f"qt{i}", name=f"qt{i}")
              for i in range(2)]
        kt = [const.tile([128, S], F32R, tag=f"kt{i}", name=f"kt{i}")
              for i in range(2)]
        # V with a ones column per head: [k, 4*65]; bf16 (AV stationary)
        vsb = [const.tile([128, HPC * 65], att_np, tag=f"v{i}", name=f"v{i}")
               for i in range(NK)]
        # normalized attention outputs, head-pairs stacked on partitions
        ust = [const.tile([128, S], F32R, tag=f"ust{i}", name=f"ust{i}")
               for i in range(2)]
        # merged weights: wt[:, dk*768 + {q:0,k:256,v:512} + c]
        wt = const.tile([128, DKN * 3 * DH], F32R, tag="wt", name="wt")
        # merged Wo: wot[:, i*1024 + c] for row-half i
        wot = const.tile([128, 2 * D_MODEL], F32R, tag="wot", name="wot")
        # saturated-bucket bias constants [128, 2*HPC] (see make_in_maps)
        bcn = const.tile([128, 2 * HPC], F32, tag="bcn", name="bcn")

        ebp = ctx.enter_context(tc.tile_pool(name="expb", bufs=2))
        xtqp = ctx.enter_context(tc.tile_pool(name="xtq", bufs=2))
        esp = ctx.enter_context(tc.tile_pool(name="es", bufs=2))
        esbp = ctx.enter_context(tc.tile_pool(name="esb", bufs=4))
        rzp = ctx.enter_context(tc.tile_pool(name="rz", bufs=2))
        outp = ctx.enter_context(tc.tile_pool(name="outsb", bufs=2))
        # PSUM: pps = 2 tags x 1 buf of [128,1024] (4 banks, scores/proj),
        #       ppu = 2 tags x 2 bufs of [128,512] (4 banks, AV accum/V/out)
        pps = ctx.enter_context(tc.tile_pool(name="pps", bufs=1, space="PSUM"))
        ppu = ctx.enter_context(tc.tile_pool(name="ppu", bufs=2, space="PSUM"))

        ns = dict(const=const, ebp=ebp, xtqp=xtqp, esp=esp, esbp=esbp,
                  rzp=rzp, outp=outp, pps=pps, ppu=ppu, qt=qt, kt=kt,
                  vsb=vsb, ust=ust, wt=wt, wot=wot, bcn=bcn, ins=ins,
                  outs=outs, att_np=att_np)

        if reps == 1:
            _emit_body(tc, nc, ns)
        else:
            with tc.For_i(0, reps):
                _emit_body(tc, nc, ns)


def _emit_body(tc, nc, ns):
    (const, ebp, xtqp, esp, esbp, rzp, outp, pps, ppu, qt, kt, vsb, ust, wt,
     wot, bcn, ins, outs, att_np) = (ns[k] for k in (
        "const", "ebp", "xtqp", "esp", "esbp", "rzp", "outp", "pps", "ppu",
        "qt", "kt", "vsb", "ust", "wt", "wot", "bcn", "ins", "outs",
        "att_np"))

    xt_d = ins["xt"].bitcast(F32R)        # [1024, 2048]
    wqkv_d = ins["wqkv"].bitcast(F32R)    # [1024, 768]
    wo_d = ins["wo"].bitcast(F32R)        # [256, 1024]
    eb_d = ins["expb"]                    # [HPC, 128, TBL] ATT_DT
    out_d = outs["out"]                   # [2048, 1024] f32

    xtr = xt_d.rearrange("(dk p) q -> p dk q", dk=DKN)     # [128, 8, 2048]

    # ---- weight + bias-table loads (one 3MB, one 1MB, 4x 1MB DMAs)
    nc.sync.dma_start(
        out=wt.rearrange("p (dk c) -> p dk c", dk=DKN),
        in_=wqkv_d.rearrange("(dk p) c -> p dk c", dk=DKN))
    nc.sync.dma_start(
        out=wot.rearrange("p (i c) -> p i c", i=2),
        in_=wo_d.rearrange("(i p) c -> p i c", i=2))
    nc.sync.dma_start(out=bcn, in_=ins["bcn"])
    ebs = {}
    for j in range(2):
        eb = ebp.tile([128, TBL], att_np, tag=f"eb{j}", name=f"eb0_{j}")
        nc.sync.dma_start(out=eb, in_=eb_d[j])
        ebs[(0, j)] = eb

    def wq_s(dk):
        return wt[:, dk * 768:dk * 768 + 256]

    def wk_s(dk):
        return wt[:, dk * 768 + 256:dk * 768 + 512]

    def wv_s(dk):
        return wt[:, dk * 768 + 512:dk * 768 + 768]

    # ================= phase 1: fused QKV projections =================
    # xT streamed once as [128, 4096] tiles; V's stationary chunks are
    # slices of the same tile (kc = 4*qb + t covers this q block).
    for _p1rep in range(2 if PROBE_P1X2 else 1):
      for qb in range(NQ):
        xtq = xtqp.tile([128, DKN * QB], F32R, tag="xtq", name=f"xtq{qb}")
        nc.sync.dma_start(
            out=xtq.rearrange("p (dk q) -> p dk q", dk=DKN),
            in_=xtr[:, :, qb * QB:(qb + 1) * QB])
        if PROBE_DMA2:
            nc.sync.dma_start(
                out=xtq.rearrange("p (dk q) -> p dk q", dk=DKN),
                in_=xtr[:, :, qb * QB:(qb + 1) * QB])

        pqm = [pps.tile([128, QB], F32, tag=f"s{m}", name=f"pq{m}_{qb}")
               for m in range(2)]
        pkm = [pps.tile([128, QB], F32, tag=f"s{m}", name=f"pk{m}_{qb}")
               for m in range(2)]
        pv2 = [ppu.tile([128, QB], F32, tag=f"u{i}", name=f"pv2_{qb}_{i}")
               for i in range(2)]
        for dk in range(DKN):
            xs = xtq[:, dk * QB:(dk + 1) * QB]
            first, last = dk == 0, dk == DKN - 1
            for m in range(2):
                nc.tensor.matmul(
                    pqm[m], wq_s(dk)[:, m * 128:(m + 1) * 128], xs,
                    start=first, stop=last)
                nc.tensor.matmul(
                    pkm[m], wk_s(dk)[:, m * 128:(m + 1) * 128], xs,
                    start=first, stop=last)
            for t in range(4):
                pv = pv2[t // 2][:, (t % 2) * DH:(t % 2 + 1) * DH]
                # start only on the bank's FIRST chunk: start=True clears
                # has_written for the WHOLE bank, so the t%2==1 chunk must
                # not re-clear (it would wipe t%2==0's dk=0 partials); its
                # dk=0 write lands as overwrite via cleared has_written bits.
                nc.tensor.matmul(pv, xs[:, t * 128:(t + 1) * 128], wv_s(dk),
                                 start=(first and t % 2 == 0), stop=last)
        # evacuate: Q/K on DVE, V on ACT (both have slack in phase 1)
        for m in range(2):
            nc.vector.tensor_copy(
                out=qt[m][:, qb * QB:(qb + 1) * QB], in_=pqm[m])
            nc.vector.tensor_copy(
                out=kt[m][:, qb * QB:(qb + 1) * QB], in_=pkm[m])
        for t in range(4):
            kc = 4 * qb + t
            v3 = vsb[kc].rearrange("p (h c) -> p h c", h=HPC)
            pv = pv2[t // 2][:, (t % 2) * DH:(t % 2 + 1) * DH]
            nc.scalar.copy(
                out=v3[:, :, 0:64],
                in_=pv.rearrange("p (h c) -> p h c", h=HPC))
            nc.vector.memset(v3[:, :, 64:65], 1.0)

    # ================= phase 2+3: attention =================
    # Heads in pairs (2hp, 2hp+1): score matmuls use disjoint PE row
    # groups (rows 0-63 / 64-127).  Score tiles are single-bank [128,512]
    # in 4 PSUM slots (2 tags x 2 bufs) so scores(kc+1) overlaps exp(kc);
    # the attn@V matmuls are emitted ONE kc LATE so the in-order PE queue
    # never stalls waiting for the exp->mul chain of the current kc.
    for hp in range(2 if PROBE_PHASES != "p1" else 0):
        if hp == 0:
            # prefetch pair 1's tables during pair 0's compute
            for j in range(2):
                eb = ebp.tile([128, TBL], att_np, tag=f"eb{j}",
                              name=f"eb1_{j}")
                nc.sync.dma_start(out=eb, in_=eb_d[2 + j])
                ebs[(1, j)] = eb
        for qb in range(NQ):
            pus = [ppu.tile([128, QB], F32, tag=f"u{j}",
                            name=f"pu{j}_{hp}_{qb}") for j in range(2)]
            esb_prev = None
            for kc in range(NK + 1):
                if kc < NK:
                    base = (TBL - S) - kc * 128 + qb * QB
                    cbk = _const_tile_bucket(kc, qb)
                    sjt = [pps.tile([128, QB], F32, tag=f"s{j}",
                                    name=f"s{j}_{hp}_{qb}_{kc}")
                           for j in range(2)]
                    for j in range(2):
                        prow = slice(j * 64, j * 64 + 64)
                        for _r in range(2 if PROBE_PE2 else 1):
                            nc.tensor.matmul(
                                sjt[j],
                                kt[hp][prow, kc * 128:(kc + 1) * 128],
                                qt[hp][prow, qb * QB:(qb + 1) * QB],
                                start=True, stop=True)
                    esb_cur = []
                    for j in range(2):
                        esb = esbp.tile([128, QB], att_np, tag=f"esb{j}",
                                        name=f"esb{j}_{hp}_{qb}_{kc}")
                        if cbk is not None:
                            # whole tile maps to one saturated rel-pos
                            # bucket: fold the bias into the exp (free ACT
                            # bias operand) and skip the DVE multiply
                            assert cbk in (15, 31), cbk
                            col = (0 if cbk == 15 else HPC) + hp * 2 + j
                            nc.scalar.activation(
                                out=esb, in_=sjt[j], func=AF.Exp,
                                bias=bcn[:, col:col + 1])
                            if PROBE_ACT2:
                                nc.scalar.activation(
                                    out=esb, in_=sjt[j], func=AF.Exp,
                                    bias=bcn[:, col:col + 1])
                        else:
                            es = esp.tile([128, QB], att_np, tag=f"es{j}",
                                          name=f"es{j}_{hp}_{qb}_{kc}")
                            nc.scalar.activation(out=es, in_=sjt[j],
                                                 func=AF.Exp)
                            if PROBE_ACT2:
                                # SBUF-neutral: double-write the tile (WAW)
                                nc.scalar.activation(out=es, in_=sjt[j],
                                                     func=AF.Exp)
                            for _r in range(2 if PROBE_MUL2 else 1):
                                nc.vector.tensor_mul(
                                    esb, es, ebs[(hp, j)][:, base:base + QB])
                        esb_cur.append(esb)
                if kc >= 1:
                    kcp = kc - 1
                    for j in range(2):
                        h = hp * 2 + j
                        nc.tensor.matmul(
                            pus[j][0:65, :],
                            vsb[kcp][:, h * 65:(h + 1) * 65], esb_prev[j],
                            start=(kcp == 0), stop=(kcp == NK - 1))
                if kc < NK:
                    esb_prev = esb_cur
            # normalize U[d, q] / Z[q]; Z = row 64 of pu
            for j in range(2):
                rz = rzp.tile([1, QB], F32, tag=f"rz{j}", name=f"rz{j}_{qb}")
                nc.vector.reciprocal(out=rz, in_=pus[j][64:65, :])
                rzb = rzp.tile([64, QB], F32, tag=f"rzb{j}",
                               name=f"rzb{j}_{qb}")
                nc.gpsimd.partition_broadcast(rzb, rz, channels=64)
                if PROBE_NORM2:
                    # SBUF-neutral: double-write the same tiles (WAW)
                    nc.vector.reciprocal(out=rz, in_=pus[j][64:65, :])
                    nc.gpsimd.partition_broadcast(rzb, rz, channels=64)
                if j == 0:
                    nc.vector.tensor_mul(
                        ust[hp][0:64, qb * QB:(qb + 1) * QB],
                        pus[j][0:64, :], rzb)
                else:
                    # DVE lanes are partition-locked; write via a [64,512]
                    # staging tile then DMA to rows 64-127
                    stg = rzp.tile([64, QB], F32R, tag="stg",
                                   name=f"stg{hp}_{qb}")
                    nc.vector.tensor_mul(stg, pus[j][0:64, :], rzb)
                    nc.sync.dma_start(
                        out=ust[hp][64:128, qb * QB:(qb + 1) * QB],
                        in_=stg)

    # ================= phase 4: output projection =================
    # two 128-row chunks per [128,2048] staging tile -> 8x 1MB out DMAs
    if PROBE_PHASES != "all":
        # timing-only variants: write a dummy output (no ust dependency)
        for qg in range(S // 256):
            ob = outp.tile([128, 2 * D_MODEL], F32, tag="ob", name=f"ob{qg}")
            nc.vector.memset(ob, 0.0)
            nc.sync.dma_start(
                out=out_d[qg * 256:(qg + 1) * 256, :].rearrange(
                    "(h p) c -> p h c", h=2),
                in_=ob.rearrange("p (h c) -> p h c", h=2))
        return
    for _p4rep in range(2 if PROBE_P4X2 else 1):
      for qg in range(S // 256):
        ob = outp.tile([128, 2 * D_MODEL], F32, tag="ob", name=f"ob{qg}")
        for half in range(2):
            qc = 2 * qg + half
            for e in range(2):
                po = ppu.tile([128, QB], F32, tag=f"u{e}",
                              name=f"po{qc}_{e}")
                for i in range(2):
                    nc.tensor.matmul(
                        po,
                        ust[i][:, qc * 128:(qc + 1) * 128],
                        wot[:, i * D_MODEL + e * QB:i * D_MODEL + (e + 1) * QB],
                        start=(i == 0), stop=(i == 1))
                nc.vector.tensor_copy(
                    out=ob[:, half * D_MODEL + e * QB:half * D_MODEL + (e + 1) * QB],
                    in_=po)
        nc.sync.dma_start(
            out=out_d[qg * 256:(qg + 1) * 256, :].rearrange(
                "(h p) c -> p h c", h=2),
            in_=ob.rearrange("p (h c) -> p h c", h=2))

    if DEBUG_DUMP:
        nc.sync.dma_start(out=outs["dbg_qt0"].bitcast(F32R), in_=qt[0])
        nc.sync.dma_start(out=outs["dbg_kt0"].bitcast(F32R), in_=kt[0])
        nc.sync.dma_start(out=outs["dbg_v0"], in_=vsb[0])
        nc.sync.dma_start(out=outs["dbg_ust0"].bitcast(F32R), in_=ust[0])
        nc.sync.dma_start(out=outs["dbg_wt"].bitcast(F32R), in_=wt)


# ------------------------------------------------------------- build + run
def _build(reps=1):
    key = ("nc", reps, PROBE_DMA2, PROBE_ACT2, PROBE_NORM2, PROBE_P1X2,
           PROBE_P4X2, PROBE_MUL2, PROBE_PE2, PROBE_PHASES, DEBUG_DUMP)
    if key in _cache:
        return _cache[key]
    nc = bacc.Bacc("TRN2", target_bir_lowering=False, debug=False)
    ins = {
        "xt": nc.dram_tensor("xt", [D_MODEL, S], F32R, kind="ExternalInput").ap(),
        "wqkv": nc.dram_tensor("wqkv", [D_MODEL, 3 * DH], F32R,
                               kind="ExternalInput").ap(),
        "wo": nc.dram_tensor("wo", [DH, D_MODEL], F32R, kind="ExternalInput").ap(),
        "expb": nc.dram_tensor("expb", [HPC, KC, TBL], ATT_DT,
                               kind="ExternalInput").ap(),
        "bcn": nc.dram_tensor("bcn", [128, 2 * HPC], F32,
                              kind="ExternalInput").ap(),
    }
    outs = {
        "out": nc.dram_tensor("out", [S, D_MODEL], F32, kind="ExternalOutput").ap(),
    }
    if DEBUG_DUMP:
        outs["dbg_qt0"] = nc.dram_tensor(
            "dbg_qt0", [128, S], F32, kind="ExternalOutput").ap()
        outs["dbg_kt0"] = nc.dram_tensor(
            "dbg_kt0", [128, S], F32, kind="ExternalOutput").ap()
        outs["dbg_v0"] = nc.dram_tensor(
            "dbg_v0", [128, HPC * 65], ATT_DT, kind="ExternalOutput").ap()
        outs["dbg_ust0"] = nc.dram_tensor(
            "dbg_ust0", [128, S], F32, kind="ExternalOutput").ap()
        outs["dbg_wt"] = nc.dram_tensor(
            "dbg_wt", [128, DKN * 3 * DH], F32, kind="ExternalOutput").ap()
    with tile.TileContext(nc) as tc:
        mha_body(tc, outs, ins, reps=reps)
    nc.compile()
    _cache[key] = nc
    return nc


TRACE = False
LAST = {}


def kernel(inputs, Wq, Wk, Wv, Wo, rel_emb):
    nc = _build()
    in_maps = make_in_maps(inputs, Wq, Wk, Wv, Wo, rel_emb)

    res = run_bass_kernel_spmd(
        nc, in_maps, core_ids=list(range(N_CORES)), trace=TRACE)
    LAST["res"] = res

    out = np.zeros((B, S, D_MODEL), dtype=np.float64)
    for c in range(N_CORES):
        b = c // (N_CORES // B)
        out[b] += res.results[c]["out"].astype(np.float64)
    return out.astype(np.float32)
